# revision 1
# baseline (speedup 1.0000x reference)
"""Causal self-attention (B=4, T=2048, E=512, H=8) on 8 TRN2 NeuronCores.

Sharding: core c -> (batch b = c//2, head-group hg = c%2, 4 heads each).
Host sums the two partial projection outputs per batch.

Design (v2):
- qkv proj per token group tg; q/k feature-major (qkT), v token-major + ones
  column (v4) for softmax denominators.
- Scores as S^T = K^T.T-style matmuls with K=64 contraction; the two heads of
  a pair sit at partitions 0-63 / 64-127 so their score matmuls occupy
  disjoint PE row groups and run concurrently.
- exp split between ScalarE (activation Exp) and VectorE (Schraudolph fp16
  bit-trick: i16 = round(x*1477.32 + 15301), bitcast to f16; ~3% per-element,
  validated < 2e-2 end-to-end).
- PV with v stationary (LDWEIGHTS 65 cols, hidden): yraw^T[65, 512] per
  (head, qg) in PSUM; row 64 = softmax denominator.
- Normalize: denom -> f16 sbuf -> K=1 ones-matmul broadcast across 64
  partitions -> reciprocal_approx_fast -> multiply into yT (no transposes).
- Output proj: 4 accumulating K=64 matmuls per token chunk (even/odd head
  yT tiles); bias pre-broadcast into sbuf once (also absorbs the v-proj bias,
  folded on host into bpp = bp + bv @ Wp).
"""

from contextlib import ExitStack

import numpy as np

import concourse.bass as bass
import concourse.mybir as mybir
import concourse.tile as tile
from concourse import bacc
from concourse.bass import ts
from concourse.bass_utils import run_bass_kernel_spmd

f32 = mybir.dt.float32
f16 = mybir.dt.float16
i16 = mybir.dt.int16
FA = mybir.ActivationFunctionType
MUL = mybir.AluOpType.mult
ADD = mybir.AluOpType.add

B, T, E = 4, 2048, 512
H, D = 8, 64
HPC = 4              # heads per core
EC = HPC * D         # 256
P = 128
NCORES = 8
TQ = T // P          # 16 token chunks
NQG = T // 512       # 4 query groups
EO = E // P          # 4 contraction subtiles
SCALE = 1.0 / np.sqrt(D)

# Schraudolph fp16 fast-exp constants (round-half-even on DVE f32->i16)
A16 = float(2.0**10 / np.log(2.0))
B16 = 15360.0 - 59.0

# dve8: of every 8 exp tiles, this many go to the DVE (rest ScalarE)
CFG = {"dve8": 4, "pS_bufs": 2, "pG_bufs": 2, "expS_bufs": 6, "xT_bufs": 2, "no_ilv": 0, "gps_mask": 1, "exp_pat": "alt"}


def _emit(tc, ctx, aps, reps=1):
    nc = tc.nc
    z = aps["z"]

    cst = ctx.enter_context(tc.tile_pool(name="cst", bufs=1))
    wqk_sb = cst.tile([P, EO, 2 * EC], f16)
    for eo in range(EO):
        nc.sync.dma_start(wqk_sb[:, eo, :], aps["wqk"][:, eo, :])
    bqk_sb = cst.tile([P, 4], f32)
    nc.sync.dma_start(bqk_sb, aps["bqk"])
    wv_sb = cst.tile([P, EO, EC], f16)
    nc.sync.dma_start(wv_sb, aps["wv"])
    wp_sb = cst.tile([P, 2, E], f16)
    nc.sync.dma_start(wp_sb, aps["wp"])
    bpp_sb = cst.tile([1, E], f16)
    nc.sync.dma_start(bpp_sb, aps["bpp"])
    ones_sb = cst.tile([1, P], f16)
    nc.sync.dma_start(ones_sb, aps["ones"])
    mask_sb = cst.tile([P, 4, 512], f16)
    nc.sync.dma_start(mask_sb, aps["mask"])
    ones2 = cst.tile([P, 64], f16)
    nc.vector.memset(ones2, 1.0)

    big = ctx.enter_context(tc.tile_pool(name="big", bufs=1))
    qkT = big.tile([P, EO, T], f16)          # sub 0-1: q^T, 2-3: k^T
    v_sb = big.tile([P, TQ, HPC * 65], f16)  # per head: 64 v cols + ones col
    v4 = v_sb.rearrange("p t (h c) -> p t h c", c=65)
    yT = big.tile([P, 2, T], f16)            # [0:64]: heads 0,2; [64:128]: 1,3
    bp_bc = big.tile([P, E], f16)            # proj bias broadcast to 128 rows

    xTp = ctx.enter_context(tc.tile_pool(name="xTp", bufs=CFG["xT_bufs"]))
    pS = ctx.enter_context(tc.tile_pool(name="pS", bufs=CFG["pS_bufs"], space="PSUM"))
    pV = ctx.enter_context(tc.tile_pool(name="pV", bufs=1, space="PSUM"))
    pG = ctx.enter_context(tc.tile_pool(name="pG", bufs=CFG["pG_bufs"], space="PSUM"))
    expSp = ctx.enter_context(tc.tile_pool(name="expSp", bufs=CFG["expS_bufs"]))
    denp = ctx.enter_context(tc.tile_pool(name="denp", bufs=2))
    rcbp = ctx.enter_context(tc.tile_pool(name="rcbp", bufs=2))
    yos = ctx.enter_context(tc.tile_pool(name="yos", bufs=2))
    zout = ctx.enter_context(tc.tile_pool(name="zout", bufs=3))

    # one-time: broadcast proj bias to all 128 partitions
    pb = pG.tile([P, E], f32, tag="g")
    nc.tensor.matmul(pb, lhsT=ones_sb, rhs=bpp_sb, start=True, stop=True)
    nc.vector.tensor_copy(bp_bc, pb)

    exp_ctr = [0]

    def emit_exp(pSt, out_ap):
        """exp of one [128, n*512] psum tile into expS f16, engine by knob.
        exp_pat=alt: alternate DVE/ScalarE per tile so the two heads of each
        chunk drain on different engines in parallel (same 50/50 split)."""
        if CFG.get("exp_pat") == "alt":
            use_dve = exp_ctr[0] % 2 == 0
        else:
            use_dve = (exp_ctr[0] % 8) < CFG["dve8"]
        exp_ctr[0] += 1
        if use_dve:
            nc.vector.tensor_scalar(
                out_ap.bitcast(i16), pSt, A16, B16, MUL, ADD
            )
        else:
            nc.scalar.activation(out_ap, pSt, FA.Exp)

    def phase1_chunks(xT, tg):
        """Generator: qkv projection for token group tg, yield per chunk.
        All q/k first (so dependent score matmuls unblock early), then v."""
        for jc in range(4):
            pq = pG.tile([P, 512], f32, tag="g")
            for eo in range(EO):
                nc.tensor.matmul(
                    pq,
                    lhsT=wqk_sb[:, eo, ts(jc, P)],
                    rhs=xT[:, eo, ts(tg, 512)],
                    start=(eo == 0),
                    stop=(eo == EO - 1),
                )
            nc.scalar.activation(
                qkT[:, jc, ts(tg, 512)], pq, FA.Identity, bias=bqk_sb[:, jc : jc + 1]
            )
            yield
        for j in range(4):
            tq = 4 * tg + j
            pv = pG.tile([P, 512], f32, tag="g")
            for eo in range(EO):
                nc.tensor.matmul(
                    pv[:, :EC],
                    lhsT=xT[:, eo, ts(tq, P)],
                    rhs=wv_sb[:, eo, :],
                    start=(eo == 0),
                    stop=(eo == EO - 1),
                )
            nc.scalar.activation(
                v4[:, tq, :, 0:64],
                pv[:, :EC].rearrange("p (h c) -> p h c", c=64),
                FA.Copy,
            )
            yield

    def phase1(xT, tg):
        _drain(phase1_chunks(xT, tg))

    def load_xT():
        xT = xTp.tile([P, EO, T], f16, tag="x", name="xT_sb")
        for eo in range(0, EO, 2):
            for th in range(4):
                nc.sync.dma_start(
                    xT[:, eo : eo + 2, ts(th, T // 4)],
                    aps["xT"][:, eo : eo + 2, ts(th, T // 4)],
                )
        return xT

    def scores_chunks(qg, expS4):
        """Generator: S^T block-pairs + exp for all 4 heads; yields between
        chunks so PE-dense work can interleave. expS4 = 4 expS tiles.

        Diagonal blocks (kb = 4qg+j) are causally trimmed to the valid query
        range [128j, 512) and stored LEFT-PACKED in their expS slot: stored
        col c' = query 128j + c'. Within a slot only [0:512-128j] is live;
        the leading 128 stored cols carry the c' >= p staircase mask."""
        nb = 4 * qg + 4
        for pair in range(2):
            heads = (2 * pair, 2 * pair + 1)
            q_sub = pair
            k_sub = 2 + pair
            for g0 in range(0, nb, 2):
                # block widths; psum/expS stay slot-aligned (offset kk*512),
                # the exp instruction spans [0 : 512 + w1] including any
                # never-read junk between block 0's live end and block 1
                w = []
                off = [0, 512]
                for kk in range(2):
                    j = g0 + kk - 4 * qg  # diag index, <=0 for full blocks
                    w.append(512 - 128 * j if j > 0 else 512)
                o = 512 + w[1]
                tiles = []
                for h in heads:
                    hp = (h % 2) * 64
                    pSt = pS.tile([P, 1024], f32, tag="s", name=f"pS_{qg}_{h}_{g0}")
                    tiles.append(pSt)
                    for kk in range(2):
                        kb = g0 + kk
                        j = max(kb - 4 * qg, 0)
                        nc.tensor.matmul(
                            pSt[:, off[kk] : off[kk] + w[kk]],
                            lhsT=qkT[hp : hp + 64, k_sub, ts(kb, P)],
                            rhs=qkT[
                                hp : hp + 64, q_sub,
                                qg * 512 + 128 * j : (qg + 1) * 512,
                            ],
                            start=True,
                            stop=True,
                        )
                for j, h in enumerate(heads):
                    ef = expS4[2 * pair + j].rearrange("p a b -> p (a b)")
                    emit_exp(
                        tiles[j][:, 0 : o], ef[:, g0 * 512 : g0 * 512 + o]
                    )
                yield
            # staircase mask on the leading 128 stored cols of the 4
            # (left-packed) diagonal blocks of each head
            for j in range(2):
                mask_eng = nc.gpsimd if CFG["gps_mask"] else nc.vector
                mask_eng.tensor_tensor(
                    expS4[2 * pair + j][:, 4 * qg : 4 * qg + 4, 0:128],
                    expS4[2 * pair + j][:, 4 * qg : 4 * qg + 4, 0:128],
                    mask_sb[:, 0:1, 0:128].to_broadcast((P, 4, 128)),
                    MUL,
                )
            yield

    def pv_pair(qg, pair, expS4, sgen):
        """PV for the two heads of `pair` -> pv2 psum [65, 2, 512].
        Pulls one scores chunk from sgen per kb step to interleave."""
        nb = 4 * qg + 4
        heads = (2 * pair, 2 * pair + 1)
        pv2 = pV.tile([65, 2, 512], f32, tag="v", name=f"pv2_{qg}_{pair}")
        for kb in range(nb):
            dj = max(kb - 4 * qg, 0)   # diag offset: slot is left-packed
            w = 512 - 128 * dj
            for j, h in enumerate(heads):
                nc.tensor.matmul(
                    pv2[:, j, 128 * dj : 512],
                    lhsT=v4[:, kb, h, :],
                    rhs=expS4[2 * pair + j][:, kb, 0:w],
                    start=(kb == 0),
                    stop=(kb == nb - 1),
                )
            _pull(sgen)
        return pv2

    def normalize_den(qg, pair, pv2):
        """denominator row -> f16 sbuf (DVE; overlap PE work before fin)."""
        den = denp.tile([P, 2, 512], f16, tag="d", name=f"den_{qg}_{pair}")
        nc.vector.tensor_copy(den[64:65, :, :], pv2[64:65, :, :])
        return den

    def normalize_fin(qg, pair, pv2, den):
        """bcast -> reciprocal -> scale into yT tiles."""
        heads = (2 * pair, 2 * pair + 1)
        for j, h in enumerate(heads):
            bc = pG.tile([P, 512], f32, tag="g")
            nc.tensor.matmul(
                bc[0:64, :],
                lhsT=ones2[64:65, :],
                rhs=den[64:65, j, :],
                start=True,
                stop=True,
            )
            rcb = rcbp.tile([64, 512], f32, tag="r")
            nc.vector.reciprocal_approx_fast(rcb, bc[0:64, :])
            if h % 2 == 0:
                nc.vector.tensor_tensor(
                    yT[0:64, h // 2, ts(qg, 512)], pv2[0:64, j, :], rcb, MUL
                )
            else:
                yo = yos.tile([64, 512], f16, tag="o", name=f"yo_{qg}_{h}")
                nc.vector.tensor_tensor(yo, pv2[0:64, j, :], rcb, MUL)
                nc.sync.dma_start(yT[64:128, h // 2, ts(qg, 512)], yo)

    def proj_z(qg, sgen):
        """output projection for the 4 token chunks of query group qg."""
        for tq in range(4 * qg, 4 * qg + 4):
            pz = pG.tile([P, 512], f32, tag="g", name=f"pz_{tq}")
            for e in range(2):
                nc.tensor.matmul(
                    pz,
                    lhsT=yT[:, e, ts(tq, P)],
                    rhs=wp_sb[:, e, :],
                    start=(e == 0),
                    stop=(e == 1),
                )
            zt = zout.tile([P, E], f16, tag="z", name=f"zt_{tq}")
            nc.vector.tensor_tensor(zt, pz, bp_bc, ADD)
            nc.sync.dma_start(z[ts(tq, P), :], zt)
            _pull(sgen)

    def _pull(gen):
        if gen is not None:
            try:
                next(gen)
            except StopIteration:
                pass

    def _drain(gen):
        if gen is not None:
            for _ in gen:
                pass

    def new_exp4(qg):
        return [
            expSp.tile([P, TQ, 512], f16, tag="e", name=f"exp{j}_{qg}")
            for j in range(4)
        ]

    from itertools import chain as _chain

    nc.vector.memset(v4[:, :, :, 64], 1.0)
    xT = load_xT()
    phase1(xT, 0)
    exp4 = new_exp4(0)
    _drain(scores_chunks(0, exp4))
    for r in range(reps):
        last = r == reps - 1
        for qg in range(NQG):
            # dense PE work for qg, interleaved with scores+exp of qg+1
            # (for qg==3: with the NEXT rep's phase1(0) + scores(0))
            if qg < NQG - 1:
                phase1(xT, qg + 1)
                exp4n = new_exp4(qg + 1)
                sgen = scores_chunks(qg + 1, exp4n)
            elif not last:
                xTn = load_xT()
                exp4n = new_exp4(0)
                sgen = _chain(phase1_chunks(xTn, 0), scores_chunks(0, exp4n))
            else:
                xTn, exp4n, sgen = None, None, None
            if sgen is not None and CFG["no_ilv"]:
                _drain(sgen)
                sgen = None
            pv2a = pv_pair(qg, 0, exp4, sgen)
            normalize_fin(qg, 0, pv2a, normalize_den(qg, 0, pv2a))
            pv2b = pv_pair(qg, 1, exp4, sgen)
            normalize_fin(qg, 1, pv2b, normalize_den(qg, 1, pv2b))
            if qg > 0:
                proj_z(qg - 1, sgen)
            _drain(sgen)
            exp4 = exp4n
            if qg == NQG - 1 and not last:
                xT = xTn
        proj_z(NQG - 1, None)


def build(reps=1):
    nc = bacc.Bacc("TRN2", target_bir_lowering=False, debug=False)
    aps = {
        "xT": nc.dram_tensor("xT", [P, EO, T], f16, kind="ExternalInput").ap(),
        "wqk": nc.dram_tensor("wqk", [P, EO, 2 * EC], f16, kind="ExternalInput").ap(),
        "bqk": nc.dram_tensor("bqk", [P, 4], f32, kind="ExternalInput").ap(),
        "wv": nc.dram_tensor("wv", [P, EO, EC], f16, kind="ExternalInput").ap(),
        "wp": nc.dram_tensor("wp", [P, 2, E], f16, kind="ExternalInput").ap(),
        "bpp": nc.dram_tensor("bpp", [1, E], f16, kind="ExternalInput").ap(),
        "ones": nc.dram_tensor("ones", [1, P], f16, kind="ExternalInput").ap(),
        "mask": nc.dram_tensor("mask", [P, 4, 512], f16, kind="ExternalInput").ap(),
        "z": nc.dram_tensor("z", [T, E], f16, kind="ExternalOutput").ap(),
    }
    with tile.TileContext(nc) as tc, ExitStack() as ctx:
        _emit(tc, ctx, aps, reps=reps)
    nc.compile()
    return nc


def make_in_maps(x, c_attn_w, c_attn_b, c_proj_w, c_proj_b):
    x = np.asarray(x, np.float32)
    W = np.asarray(c_attn_w, np.float32)
    bW = np.asarray(c_attn_b, np.float32)
    Wp = np.asarray(c_proj_w, np.float32)
    bp = np.asarray(c_proj_b, np.float32)

    ones = np.ones((1, P), np.float16)
    # mask[p, j, c] = 1 iff query col c >= key row p + 128*j (causal staircase)
    pp = np.arange(P)[:, None, None]
    jj = np.arange(4)[None, :, None]
    cc = np.arange(512)[None, None, :]
    mask = (cc >= pp + 128 * jj).astype(np.float16)
    in_maps = []
    for c in range(NCORES):
        b, hg = c // 2, c % 2
        qs = slice(hg * EC, (hg + 1) * EC)
        ks = slice(E + hg * EC, E + (hg + 1) * EC)
        vs = slice(2 * E + hg * EC, 2 * E + (hg + 1) * EC)
        wqk = np.concatenate([W[:, qs] * SCALE, W[:, ks]], axis=1)  # [512, 512]
        bqk = np.concatenate([bW[qs] * SCALE, bW[ks]])              # [512]
        xT = np.ascontiguousarray(
            x[b].T.reshape(EO, P, T).transpose(1, 0, 2)
        ).astype(np.float16)
        Wp_core = Wp[hg * EC : (hg + 1) * EC, :]                    # [256, 512]
        bpp = (bp if hg == 0 else np.zeros_like(bp)) + bW[vs] @ Wp_core
        in_maps.append({
            "xT": xT,
            "wqk": np.ascontiguousarray(
                wqk.reshape(EO, P, 2 * EC).transpose(1, 0, 2)
            ).astype(np.float16),
            "bqk": np.ascontiguousarray(bqk.reshape(4, P).T),
            "wv": np.ascontiguousarray(
                W[:, vs].reshape(EO, P, EC).transpose(1, 0, 2)
            ).astype(np.float16),
            "wp": np.ascontiguousarray(
                Wp_core.reshape(2, P, E).transpose(1, 0, 2)
            ).astype(np.float16),
            "bpp": bpp[None].astype(np.float16),
            "ones": ones,
            "mask": mask,
        })
    return in_maps


_NC_CACHE = {}


def kernel(x, c_attn_w, c_attn_b, c_proj_w, c_proj_b):
    if "nc" not in _NC_CACHE:
        _NC_CACHE["nc"] = build()
    nc = _NC_CACHE["nc"]
    in_maps = make_in_maps(x, c_attn_w, c_attn_b, c_proj_w, c_proj_b)
    res = run_bass_kernel_spmd(nc, in_maps, core_ids=list(range(NCORES)))
    out = np.empty((B, T, E), np.float32)
    for b in range(B):
        out[b] = (
            res.results[2 * b]["z"].astype(np.float32)
            + res.results[2 * b + 1]["z"].astype(np.float32)
        )
    return out

